# revision 1
# baseline (speedup 1.0000x reference)
"""2-layer GAT (DGL GATConv style) on 8 trn2 NeuronCores via Bass/Tile.

Design:
  - Edges dst-sorted on host; cores own contiguous equal node ranges
    [c*NPC, (c+1)*NPC) and all edges whose dst falls in range.
  - Phase A (replicated on every core): packed table row (f32 container)
    [feat(D bf16) | el(H f32) | er(H f32) | pad] = x @ [W | Wal | War],
    written to tabA (rows < SPLIT) / tabB (rest). Feature cols are stored
    h-innermost (packed col j <-> head j%H, dim j//H) so the per-head exp
    broadcast is a small-period AP. SPLIT is a multiple of NPC <= 32767 with
    N-SPLIT <= 32767 so all dma_gather indices fit int16.
  - Phase B (sharded): per dst-tile (128 dst nodes), gather src rows with
    dma_gather (slots: chunks 0..KA-1 from tabA, KA..K-1 from tabB; 128
    edges per chunk; pad slots idx=0/dstpos=255). er per-slot comes from a
    per-core local table er_loc (built once per layer by gathering own rows
    from tabA AND tabB; slot indices select the valid half). Then
    exp(leakyrelu(el+er)) per edge -> denominator cols, weighted messages,
    and segment-sum via one-hot S matmuls into PSUM [num | den]; normalize,
    bias, relu.
  - Between layers: transposed h (bf16) AllGather so every core can run
    phase A of layer 2.
"""

import math
import sys
from types import SimpleNamespace

import numpy as np

sys.path.insert(0, "/opt/trn_rl_repo")

from concourse import bacc, bass, mybir, tile  # noqa: E402

F32 = mybir.dt.float32
BF16 = mybir.dt.bfloat16
I32 = mybir.dt.int32
I16 = mybir.dt.int16

P = 128


def make_cfg(N=50000, E=800000, D=256, H=8, cores=8, split=None):
    HD = D // H
    NPC = N // cores
    NT = math.ceil(NPC / P)          # dst tiles per core
    NTA = math.ceil(N / P)           # phase-A node tiles (layer 1, flat)
    ROWU = ((D + 4 * H + 127) // 128) * 128   # packed row, bf16 units (256B mult)
    if split is None:
        split = NPC * min(cores, 32767 // NPC)
        split = min(split, N)
    assert split % NPC == 0 and split <= 32767 + 1 and N - split <= 32767 + 1
    return SimpleNamespace(
        N=N, E=E, D=D, H=H, HD=HD, cores=cores, NPC=NPC, NT=NT, NTA=NTA,
        ROWU=ROWU, ROWF=ROWU // 2, SPLIT=split,
        NEG=0.2,
    )


# ---------------------------------------------------------------- host prep

def perm_h_inner(D, H):
    """perm[j] = original feature index stored at packed col j (h-innermost)."""
    HD = D // H
    j = np.arange(D)
    return (j % H) * HD + j // H


def attn_cols(W, a, H):
    D = W.shape[0]
    HD = W.shape[1] // H
    return np.stack(
        [W[:, h * HD:(h + 1) * HD] @ a[h] for h in range(H)], axis=1
    )


def wrap16(flat, reps=8):
    """[num] -> [16*reps, num//16] int16 wrapped layout (idx i at [i%16, i//16]),
    replicated across the 8 gpsimd cores."""
    num = len(flat)
    assert num % 16 == 0
    a = np.zeros((16, num // 16), dtype=np.int16)
    a[np.arange(num) % 16, np.arange(num) // 16] = flat
    return np.tile(a, (reps, 1))


def prep_edges(src, dst, cfg):
    c = cfg
    order = np.argsort(dst, kind="stable")
    src_s = src[order].astype(np.int64)
    dst_s = dst[order].astype(np.int64)

    core = dst_s // c.NPC
    loc = dst_s - core * c.NPC
    lt = loc // P
    pos = loc - lt * P
    islow = src_s < c.SPLIT

    # group = (core, tile, islow) ; edges of a group get consecutive slots
    g = (core * c.NT + lt) * 2 + (1 - islow)   # low first
    order2 = np.argsort(g, kind="stable")
    src_s, dst_s, core, lt, pos, islow, g = (
        a[order2] for a in (src_s, dst_s, core, lt, pos, islow, g))

    uniq, starts = np.unique(g, return_index=True)
    start_of = np.zeros(c.cores * c.NT * 2, dtype=np.int64)
    start_of[uniq] = starts
    rank = np.arange(len(dst_s)) - start_of[g]

    counts = np.zeros(c.cores * c.NT * 2, dtype=np.int64)
    np.add.at(counts, g, 1)
    KA = int(math.ceil(counts[0::2].max() / P))
    KB = int(math.ceil(counts[1::2].max() / P)) if counts[1::2].max() > 0 else 0
    K = KA + KB

    # slot within tile: low edges fill chunks [0,KA), high fill [KA,K)
    chunk = rank // P + np.where(islow, 0, KA)
    part = rank % P

    srcA = np.zeros((c.cores, c.NT * KA * P), dtype=np.int64)
    srcB = np.zeros((c.cores, c.NT * KB * P), dtype=np.int64) if KB else None
    dstl = np.zeros((c.cores, c.NT * K * P), dtype=np.int64)
    dstpos = np.full((c.cores, P, c.NT * K), 255.0, dtype=np.float32)

    # flat slot index within the per-tile gather calls:
    # gather A of tile t covers slots i in [0, KA*128): chunk ca=i//128, p=i%128
    iA = lt * (KA * P) + (chunk * P + part)            # valid where islow
    iB = lt * (KB * P) + ((chunk - KA) * P + part) if KB else None
    low = islow
    srcA[core[low], iA[low]] = src_s[low]
    if KB:
        hi = ~islow
        srcB[core[hi], iB[hi]] = src_s[hi] - c.SPLIT
    # er slot index: per tile K chunks
    iE = lt * (K * P) + (chunk * P + part)
    half = (core * c.NPC >= c.SPLIT).astype(np.int64)  # whole core range one side
    dstl[core, iE] = (dst_s - core * c.NPC) + half * (c.NT * P)
    dstpos[core, part, lt * K + chunk] = pos

    srcA16 = np.stack([wrap16(srcA[ci]) for ci in range(c.cores)])
    srcB16 = (np.stack([wrap16(srcB[ci]) for ci in range(c.cores)])
              if KB else np.zeros((c.cores, P, 0), np.int16))
    dst16 = np.stack([wrap16(dstl[ci]) for ci in range(c.cores)])
    return srcA16, srcB16, dst16, dstpos, KA, KB


def prep_all(inputs, cfg):
    c = cfg
    perm = perm_h_inner(c.D, c.H)
    x = np.asarray(inputs["data"], np.float32)
    src = np.asarray(inputs["src"]).astype(np.int64)
    dst = np.asarray(inputs["dst"]).astype(np.int64)

    def rhs_for(W, al, ar, permute_rows):
        W = np.asarray(W, np.float64)
        Wal = attn_cols(W, np.asarray(al, np.float64), c.H)
        War = attn_cols(W, np.asarray(ar, np.float64), c.H)
        Wp = W[:, perm]
        if permute_rows:
            Wp, Wal, War = Wp[perm], Wal[perm], War[perm]
        return to_bf16(np.concatenate([Wp, Wal, War], axis=1))

    rhs1 = rhs_for(inputs["W1"], inputs["al1"], inputs["ar1"], False)
    rhs2 = rhs_for(inputs["W2"], inputs["al2"], inputs["ar2"], True)
    b1 = np.asarray(inputs["b1"], np.float32)[perm].reshape(1, c.D)
    b2 = np.asarray(inputs["b2"], np.float32)[perm].reshape(1, c.D)
    xT = to_bf16(x.T.copy())

    srcA16, srcB16, dst16, dstpos, KA, KB = prep_edges(src, dst, c)

    # er_loc build indices: row i (i in [0, NT*128)) <- own global node NPC*ci+i
    erbA, erbB = [], []
    for ci in range(c.cores):
        base = ci * c.NPC
        rows = np.arange(c.NT * P)
        ra = np.where(base < c.SPLIT, base + rows, 0)
        ra = np.clip(ra, 0, c.SPLIT - 1)
        rb = np.where(base >= c.SPLIT, base - c.SPLIT + rows, 0)
        rb = np.clip(rb, 0, max(c.N - c.SPLIT - 1, 0))
        erbA.append(wrap16(ra))
        erbB.append(wrap16(rb))

    meta = SimpleNamespace(perm=perm, KA=KA, KB=KB, K=KA + KB)
    in_maps = []
    for ci in range(c.cores):
        in_maps.append({
            "xT": xT, "rhs1": rhs1, "rhs2": rhs2, "b1row": b1, "b2row": b2,
            "srcA16": srcA16[ci], "srcB16": srcB16[ci], "dst16": dst16[ci],
            "erbA16": erbA[ci], "erbB16": erbB[ci],
            "dstpos": dstpos[ci],
        })
    return in_maps, meta


def to_bf16(a):
    import ml_dtypes
    return np.asarray(a).astype(ml_dtypes.bfloat16)


def finalize(results, cfg, meta):
    c = cfg
    parts = [results[ci]["out"][: c.NPC] for ci in range(c.cores)]
    out_p = np.concatenate(parts, axis=0)
    out = np.empty_like(out_p)
    out[:, meta.perm] = out_p
    return out


# ---------------------------------------------------------------- kernel

def build_nc(cfg, KA, KB, debug=False):
    c = cfg
    K = KA + KB
    NB = c.N - c.SPLIT               # rows in tabB
    ERR = c.NT * P                   # er_loc rows per half

    nc = bacc.Bacc("TRN2", target_bir_lowering=False, debug=debug,
                   num_devices=c.cores)

    xT = nc.declare_dram_parameter("xT", [c.D, c.N], BF16, isOutput=False)
    rhs1 = nc.declare_dram_parameter("rhs1", [c.D, c.D + 2 * c.H], BF16, isOutput=False)
    rhs2 = nc.declare_dram_parameter("rhs2", [c.D, c.D + 2 * c.H], BF16, isOutput=False)
    b1row = nc.declare_dram_parameter("b1row", [1, c.D], F32, isOutput=False)
    b2row = nc.declare_dram_parameter("b2row", [1, c.D], F32, isOutput=False)
    srcA16 = nc.declare_dram_parameter("srcA16", [P, c.NT * KA * 8], I16, isOutput=False)
    if KB:
        srcB16 = nc.declare_dram_parameter("srcB16", [P, c.NT * KB * 8], I16, isOutput=False)
    dst16 = nc.declare_dram_parameter("dst16", [P, c.NT * K * 8], I16, isOutput=False)
    erbA16 = nc.declare_dram_parameter("erbA16", [P, ERR // 16], I16, isOutput=False)
    erbB16 = nc.declare_dram_parameter("erbB16", [P, ERR // 16], I16, isOutput=False)
    dstpos = nc.declare_dram_parameter("dstpos", [P, c.NT * K], F32, isOutput=False)
    out_ext = nc.declare_dram_parameter("out", [c.NT * P, c.D], F32, isOutput=True)

    tabA = nc.dram_tensor("tabA", [c.SPLIT, c.ROWF], F32)
    tabB = nc.dram_tensor("tabB", [max(NB, 1), c.ROWF], F32)
    er_loc = nc.dram_tensor("er_loc", [2 * ERR, 64], F32)
    hT_loc = nc.dram_tensor("hT_loc", [c.D, c.NT * P], BF16)
    hT_ag = nc.dram_tensor("hT_ag", [c.cores * c.D, c.NT * P], BF16,
                           addr_space="Shared" if c.cores > 4 else "Local")

    KBLK = math.ceil(c.D / P)

    def tab_rows(row0, m):
        """Yield (tensor, local_row0, rows) pieces for global rows [row0, row0+m)."""
        out = []
        if row0 < c.SPLIT:
            mm = min(m, c.SPLIT - row0)
            out.append((tabA, row0, mm, 0))
        if row0 + m > c.SPLIT:
            s = max(row0, c.SPLIT)
            out.append((tabB, s - c.SPLIT, row0 + m - s, s - row0))
        return out

    with tile.TileContext(nc) as tc:
        with (
            tc.tile_pool(name="const", bufs=1) as constp,
            tc.tile_pool(name="lhs", bufs=4) as lhsp,
            tc.tile_pool(name="packed", bufs=3) as packedp,
            tc.tile_pool(name="gath", bufs=2) as gathp,
            tc.tile_pool(name="ert", bufs=2) as ertp,
            tc.tile_pool(name="rhsm", bufs=2) as rhsmp,
            tc.tile_pool(name="small", bufs=3) as smallp,
            tc.tile_pool(name="sel", bufs=4) as selp,
            tc.tile_pool(name="outp", bufs=3) as outp,
            tc.tile_pool(name="psA", bufs=2, space="PSUM") as psA,
            tc.tile_pool(name="psB", bufs=2, space="PSUM") as psB,
            tc.tile_pool(name="psT", bufs=2, space="PSUM") as psT,
        ):
            # ---------------- constants
            iota = constp.tile([P, P], BF16, tag="iota")
            nc.gpsimd.iota(iota[:], [[1, P]], channel_multiplier=0,
                           allow_small_or_imprecise_dtypes=True)
            from concourse.masks import make_identity
            ident = constp.tile([P, P], BF16, tag="ident")
            make_identity(nc, ident[:])

            def load_const(name, param, shape, dt):
                t = constp.tile(shape, dt, tag=name, name=name)
                nc.sync.dma_start(out=t[:], in_=param[:, :])
                return t

            srcA_sb = load_const("srcA_sb", srcA16, [P, c.NT * KA * 8], I16)
            srcB_sb = (load_const("srcB_sb", srcB16, [P, c.NT * KB * 8], I16)
                       if KB else None)
            dst_sb = load_const("dst_sb", dst16, [P, c.NT * K * 8], I16)
            erbA_sb = load_const("erbA_sb", erbA16, [P, ERR // 16], I16)
            erbB_sb = load_const("erbB_sb", erbB16, [P, ERR // 16], I16)
            dstpos_sb = load_const("dstpos_sb", dstpos, [P, c.NT * K], F32)

            rhsW = [[constp.tile([min(P, c.D - kb * P), c.D + 2 * c.H], BF16,
                                 tag=f"rhsW{l}_{kb}", name=f"rhsW{l}_{kb}")
                     for kb in range(KBLK)] for l in range(2)]
            for l, rt in enumerate([rhs1, rhs2]):
                for kb in range(KBLK):
                    kbsz = min(P, c.D - kb * P)
                    nc.sync.dma_start(out=rhsW[l][kb][:],
                                      in_=rt[kb * P: kb * P + kbsz, :])
            b_bc = [constp.tile([P, c.D], F32, tag=f"bbc{l}", name=f"bbc{l}")
                    for l in range(2)]
            for l, bt in enumerate([b1row, b2row]):
                nc.sync.dma_start(out=b_bc[l][:],
                                  in_=bt[0:1, :].to_broadcast([P, c.D]))

            # Pre-touch consts on DVE/PE-adjacent engines so tensor_scalar
            # (limited sync-wait slots) doesn't carry first-use waits.
            warm = constp.tile([P, 4], F32, tag="warm")
            warmb = warm[:].bitcast(BF16)
            nc.vector.tensor_copy(out=warm[:, 0:1], in_=dstpos_sb[:, 0:1])
            nc.vector.tensor_copy(out=warmb[:, 0:1], in_=iota[:, 0:1])

            # ---------------- phase A
            def phase_a(layer):
                rhs_t = rhsW[layer]
                if layer == 0:
                    jobs = []
                    for t in range(c.NTA):
                        m = min(P, c.N - t * P)
                        jobs.append((t * P, m,
                                     [(xT, kb * P, t * P) for kb in range(KBLK)]))
                else:
                    jobs = []
                    for blk in range(c.cores):
                        for t in range(c.NT):
                            m = min(P, c.NPC - t * P)
                            jobs.append((blk * c.NPC + t * P, m,
                                         [(hT_ag, blk * c.D + kb * P, t * P)
                                          for kb in range(KBLK)]))
                for row0, m, srcs in jobs:
                    ps = psA.tile([P, c.D + 2 * c.H], F32, tag="psA")
                    for kb in range(KBLK):
                        kbsz = min(P, c.D - kb * P)
                        lt_t = lhsp.tile([P, P], BF16, tag="lhsA")
                        hsrc, r0, c0 = srcs[kb]
                        nc.sync.dma_start(
                            out=lt_t[:kbsz, :m],
                            in_=hsrc[r0: r0 + kbsz, c0: c0 + m])
                        nc.tensor.matmul(
                            out=ps[:m, :], lhsT=lt_t[:kbsz, :m],
                            rhs=rhs_t[kb][:kbsz, :],
                            start=(kb == 0), stop=(kb == KBLK - 1))
                    pk = packedp.tile([P, c.ROWF], F32, tag="packed")
                    pkb = pk[:].bitcast(BF16)
                    nc.scalar.copy(out=pkb[:m, : c.D], in_=ps[:m, : c.D])
                    nc.vector.tensor_copy(
                        out=pk[:m, c.D // 2: c.D // 2 + 2 * c.H],
                        in_=ps[:m, c.D: c.D + 2 * c.H])
                    wcols = c.D // 2 + 2 * c.H
                    for tab, lr0, mm, po in tab_rows(row0, m):
                        nc.sync.dma_start(out=tab[lr0: lr0 + mm, :wcols],
                                          in_=pk[po: po + mm, :wcols])

            # ---------------- er_loc build (per layer)
            def build_er_loc():
                start = c.ROWF - 64
                eroff = (c.D // 2 + c.H) - start
                for half, (tab, idx_sb) in enumerate(
                        [(tabA, erbA_sb), (tabB, erbB_sb)]):
                    eb = ertp.tile([P, c.NT, 64], F32, tag="erb")
                    nc.gpsimd.dma_gather(
                        out_ap=eb[:, :, :],
                        in_ap=tab[:, start: start + 64],
                        idxs_ap=idx_sb[:, :],
                        num_idxs=ERR, num_idxs_reg=ERR,
                        elem_size=64, elem_step=c.ROWF,
                        single_packet=ERR <= 1024)
                    nc.sync.dma_start(
                        out=er_loc[half * ERR:(half + 1) * ERR, 0: c.H]
                        .rearrange("(t p) h -> p t h", p=P),
                        in_=eb[:, :, eroff: eroff + c.H])

            # ---------------- phase B
            def phase_b(layer):
                DH = c.D + c.H
                for t in range(c.NT):
                    g = gathp.tile([P, K, c.ROWF], F32, tag="gath")
                    nc.gpsimd.dma_gather(
                        out_ap=g[:, 0:KA, :], in_ap=tabA[:, :],
                        idxs_ap=srcA_sb[:, t * KA * 8:(t + 1) * KA * 8],
                        num_idxs=KA * P, num_idxs_reg=KA * P,
                        elem_size=c.ROWF, single_packet=KA * P <= 1024)
                    if KB:
                        nc.gpsimd.dma_gather(
                            out_ap=g[:, KA:K, :], in_ap=tabB[:, :],
                            idxs_ap=srcB_sb[:, t * KB * 8:(t + 1) * KB * 8],
                            num_idxs=KB * P, num_idxs_reg=KB * P,
                            elem_size=c.ROWF, single_packet=KB * P <= 1024)
                    ert = ertp.tile([P, K, 64], F32, tag="ert")
                    nc.gpsimd.dma_gather(
                        out_ap=ert[:, :, :], in_ap=er_loc[:, :],
                        idxs_ap=dst_sb[:, t * K * 8:(t + 1) * K * 8],
                        num_idxs=K * P, num_idxs_reg=K * P,
                        elem_size=64, single_packet=K * P <= 1024)
                    gb = g[:].bitcast(BF16)
                    el = g[:, :, c.D // 2: c.D // 2 + c.H]
                    ea = smallp.tile([P, K, c.H], F32, tag="eadd")
                    nc.vector.tensor_tensor(out=ea[:], in0=el,
                                            in1=ert[:, :, 0: c.H],
                                            op=mybir.AluOpType.add)
                    tmp = smallp.tile([P, K, c.H], F32, tag="lrtmp")
                    nc.vector.tensor_scalar_mul(tmp[:], ea[:], c.NEG)
                    lr = smallp.tile([P, K, c.H], F32, tag="lrout")
                    nc.vector.tensor_tensor(out=lr[:], in0=ea[:], in1=tmp[:],
                                            op=mybir.AluOpType.max)
                    rm = rhsmp.tile([P, K, DH], BF16, tag="rhsm")
                    nc.scalar.activation(
                        out=rm[:, :, c.D: DH], in_=lr[:],
                        func=mybir.ActivationFunctionType.Exp)
                    expb = rm[:, :, c.D: DH].unsqueeze(2).to_broadcast(
                        [P, K, c.HD, c.H])
                    feat4 = gb[:, :, : c.D].rearrange(
                        "p k (hd h) -> p k hd h", h=c.H)
                    out4 = rm[:, :, : c.D].rearrange(
                        "p k (hd h) -> p k hd h", h=c.H)
                    nc.vector.tensor_tensor(out=out4, in0=feat4, in1=expb,
                                            op=mybir.AluOpType.mult)
                    ps = psB.tile([P, DH], F32, tag="psB")
                    for ck in range(K):
                        S = selp.tile([P, P], BF16, tag="S")
                        nc.vector.tensor_scalar(
                            out=S[:], in0=iota[:],
                            scalar1=dstpos_sb[:, t * K + ck: t * K + ck + 1],
                            scalar2=None,
                            op0=mybir.AluOpType.is_equal)
                        nc.tensor.matmul(out=ps[:], lhsT=S[:],
                                         rhs=rm[:, ck, :],
                                         start=(ck == 0), stop=(ck == K - 1))
                    den = smallp.tile([P, c.H], F32, tag="den")
                    nc.vector.tensor_scalar_max(den[:], ps[:, c.D: DH], 1e-30)
                    rcp = smallp.tile([P, c.H], F32, tag="rcp")
                    nc.vector.reciprocal(rcp[:], den[:])
                    o1 = outp.tile([P, c.D], F32, tag="o1")
                    rcpb = rcp[:].unsqueeze(1).to_broadcast([P, c.HD, c.H])
                    ps4 = ps[:, : c.D].rearrange("p (hd h) -> p hd h", h=c.H)
                    o14 = o1[:].rearrange("p (hd h) -> p hd h", h=c.H)
                    nc.vector.tensor_tensor(out=o14, in0=ps4, in1=rcpb,
                                            op=mybir.AluOpType.mult)
                    nc.vector.tensor_tensor(out=o1[:], in0=o1[:],
                                            in1=b_bc[layer][:],
                                            op=mybir.AluOpType.add)
                    if layer == 0:
                        hb = outp.tile([P, c.D], BF16, tag="hb")
                        nc.vector.tensor_scalar_max(hb[:], o1[:], 0.0)
                        for kb in range(KBLK):
                            kbsz = min(P, c.D - kb * P)
                            pst = psT.tile([P, P], BF16, tag="psT")
                            nc.tensor.transpose(
                                out=pst[:kbsz, :],
                                in_=hb[:, kb * P: kb * P + kbsz],
                                identity=ident[:])
                            htb = outp.tile([P, P], BF16, tag="htb")
                            nc.scalar.copy(out=htb[:kbsz, :], in_=pst[:kbsz, :])
                            nc.sync.dma_start(
                                out=hT_loc[kb * P: kb * P + kbsz,
                                           t * P: t * P + P],
                                in_=htb[:kbsz, :])
                    else:
                        o2 = outp.tile([P, c.D], F32, tag="o2")
                        nc.vector.tensor_scalar_max(o2[:], o1[:], 0.0)
                        nc.sync.dma_start(out=out_ext[t * P: t * P + P, :],
                                          in_=o2[:])

            phase_a(0)
            build_er_loc()
            phase_b(0)
            nc.gpsimd.collective_compute(
                "AllGather",
                mybir.AluOpType.bypass,
                replica_groups=[list(range(c.cores))],
                ins=[hT_loc[:]],
                outs=[hT_ag[:]],
            )
            phase_a(1)
            build_er_loc()
            phase_b(1)

    nc.compile()
    return nc


# ---------------------------------------------------------------- reference

def ref_np(inputs, cfg):
    c = cfg
    x = np.asarray(inputs["data"], np.float64)
    src = np.asarray(inputs["src"]).astype(np.int64)
    dst = np.asarray(inputs["dst"]).astype(np.int64)

    def layer(x, W, al, ar, b):
        N = x.shape[0]
        feat = (x @ np.asarray(W, np.float64)).reshape(N, c.H, c.HD)
        el = np.einsum("nhd,hd->nh", feat, np.asarray(al, np.float64))
        er = np.einsum("nhd,hd->nh", feat, np.asarray(ar, np.float64))
        e = el[src] + er[dst]
        e = np.where(e > 0, e, c.NEG * e)
        m = np.full((N, c.H), -np.inf)
        np.maximum.at(m, dst, e)
        a = np.exp(e - m[dst])
        den = np.zeros((N, c.H))
        np.add.at(den, dst, a)
        alpha = a / den[dst]
        msg = feat[src] * alpha[:, :, None]
        out = np.zeros((N, c.H, c.HD))
        np.add.at(out, dst, msg)
        out = out + np.asarray(b, np.float64).reshape(1, c.H, c.HD)
        return np.maximum(out, 0).reshape(N, c.D)

    h = layer(x, inputs["W1"], inputs["al1"], inputs["ar1"], inputs["b1"])
    h = layer(h, inputs["W2"], inputs["al2"], inputs["ar2"], inputs["b2"])
    return h


# ---------------------------------------------------------------- entry point

_BUILD_CACHE = {}


def kernel(**inputs) -> np.ndarray:
    """Full-input GAT kernel: shards internally across 8 NeuronCores."""
    from concourse.bass_utils import run_bass_kernel_spmd

    cfg = make_cfg(N=50000, E=800000, D=256, H=8, cores=8)
    in_maps, meta = prep_all(inputs, cfg)
    key = (meta.KA, meta.KB)
    if key not in _BUILD_CACHE:
        _BUILD_CACHE[key] = build_nc(cfg, meta.KA, meta.KB)
    nc = _BUILD_CACHE[key]
    res = run_bass_kernel_spmd(nc, in_maps, list(range(cfg.cores)))
    results = [{"out": res.results[ci]["out"]} for ci in range(cfg.cores)]
    out = finalize(results, cfg, meta)
    return np.ascontiguousarray(out.astype(np.float32))

